# revision 20
# baseline (speedup 1.0000x reference)
"""Causal attention head (B=8, S=4096, dk=64, scale=1/dk) on 8 TRN2 NeuronCores.

Data-parallel: batch b -> core b (no collectives). Per core, flash-style
causal attention with scores in [kv, q] orientation and output in [q, d]:
  - QK^T bf16, row-packed 2x via 64x128 PE tiling (chunk pairs at
    tile_position (0,0)/(64,0) execute CONCURRENTLY in separate PE row
    groups): q^T is duplicated into both partition halves, k^T chunks
    alternate halves. ~250ns per 2x512 score columns.
  - exp(x/64) on the [128, 2, 512] PSUM pair-tiles split between ScalarE
    (LUT exp, 1/dk scale folded in) and VectorE (custom single-pass DVE
    poly op), assigned by a running-cost balancer at measured rates.
    Diagonal 128x128 triangles use a fused exp*mask DVE op.
  - PV uses the exp'd scores (bf16 SBUF) as the STATIONARY operand and
    v (+ ones column for the denominator) as MOVING: out[q=128, 65] +=
    at[kv, qblk]^T @ vp[kv, 65]. This is weight-load bound (~55ns per
    128x128 score block) - same PE cost as the [d, q] orientation - but
    the output lands directly in [q, d]: no epilogue transposes and no
    PSUM->SBUF copy, saving ~7us of Vector/PE epilogue work.
  - PV emission is delayed PIPE_DEPTH chunk-pairs behind QK so the PE
    stays continuously busy (it must not stall or it drops out of the
    2.4GHz p-state); epilogue is reciprocal of the ones column + 4
    normalizes (scalar.mul/vector.tensor_scalar by running cost) + DMA.
All PSUM accumulation for a superblock lives in one 2KB bank; only the
first matmul into the bank carries start=True (start marks the whole
bank pending-zero, so interleaved per-qblock groups rely on pending
bytes making their first write a fresh write).
Superblocks are processed largest-first so the pipeline is deep from the
start and the shallow ones drain at the end.

Host-side shard packing per batch (layout only; all math is on-device):
  qtp [128, 4096] bf16 : q^T duplicated into both partition halves
  ktp [128, 16, 128] bf16 : k^T chunk 2m in partitions 0-63, 2m+1 in 64-127
  vp  [128, 32, 65] bf16 : v chunks (kv on partitions) + ones column
"""

import numpy as np
import ml_dtypes
from collections import deque

B, S, DK = 8, 4096, 64
QB = 512           # q superblock width (PSUM bank = 512 fp32)
KB = 128           # kv chunk (partition dim)
NK = S // KB       # 32 kv chunks
NQ = S // QB       # 8 q superblocks
TR = 2             # kv chunks per QK psum tile (2 PSUM banks, 3 bufs)
PIPE_DEPTH = 3     # chunk-pairs of PV delay behind QK on the PE stream
PV_BURST = 2       # pairs of PV drained back-to-back (fewer PE mode flips)

_cache = {}

# exp(x/64) ~= ((c0 + c1*x + c2*x^2)^2)^2)^2  (quadratic fit of exp(x/512)
# on |x|<=64, then 3 squarings). Max rel err ~7e-4 for |x|<=64.
EXP_C0, EXP_C1, EXP_C2 = 1.0, 0.001956942, 1.909212e-06

# greedy engine-balance cost model (ns): per-column rate + per-instr
# overhead, from measured hw traces (both engines run ~1x per column here)
RATE_S, OVH_S = 1.00, 180.0     # ScalarE activation
RATE_V, OVH_V = 1.15, 150.0     # VectorE custom DVE


def _pin_sha(op):
    import re

    for ver in ("v3",):
        try:
            op.compile(ver)
        except ValueError as e:
            m = re.search(r'uops_sha\["' + ver + r'"\]="([0-9a-f]+)"', str(e))
            if not m:
                raise
            op.uops_sha[ver] = m.group(1)
            op.compile(ver)


def _register_exp_ops():
    """Custom single-pass DVE ops:
    EXP_P8_ANT:  out = sq(sq(sq(c2*x^2 + c1*x + c0)))          ~ exp(x/64)
    EXP_P8M_ANT: out = sq(sq(sq(c2*x^2 + c1*x + c0))) * in1    fused mask
    """
    from concourse import dve_ops
    from concourse.dve_spec import Spec, Src0, Src1, C0, C1, C2, sq

    have = {o.name: o for o in dve_ops.OPS}
    if "EXP_P8_ANT" in have:
        return have["EXP_P8_ANT"], have["EXP_P8M_ANT"]
    poly = sq(sq(sq((Src0 * C2 + C1) * Src0 + C0)))

    def ref(in0, in1, s0, s1, imm2):
        return ((((in0 * imm2 + s1) * in0 + s0) ** 2) ** 2) ** 2

    spec_e = Spec(body=poly, reference=ref)
    spec_m = Spec(
        body=poly * Src1,
        reference=lambda in0, in1, s0, s1, imm2: ref(in0, in1, s0, s1, imm2)
        * in1,
    )
    ops = []
    for name, spec in (("EXP_P8_ANT", spec_e), ("EXP_P8M_ANT", spec_m)):
        op = dve_ops.DveOp(name, spec, subdim=False, uops_sha={})
        dve_ops.OPS.append(op)
        dve_ops.CUSTOM_DVE_SPECS[name] = spec
        dve_ops._SUB_OPCODE_FOR_NAME[name] = (
            max(dve_ops._SUB_OPCODE_FOR_NAME.values()) + 1
        )
        _pin_sha(op)
        ops.append(op)
    return ops[0], ops[1]


def _build():
    from concourse.bacc import Bacc
    from concourse import tile
    import concourse.mybir as mybir

    exp_op, expm_op = _register_exp_ops()

    f32 = mybir.dt.float32
    bf16 = mybir.dt.bfloat16

    nc = Bacc(None, target_bir_lowering=False)
    qt_d = nc.dram_tensor("qtp", [DK, S], bf16, kind="ExternalInput")
    kt_d = nc.dram_tensor("ktp", [DK, NK, KB], bf16, kind="ExternalInput")
    vp_d = nc.dram_tensor("vp", [KB, NK, DK + 1], bf16, kind="ExternalInput")
    out_d = nc.dram_tensor("out", [S, DK], f32, kind="ExternalOutput")

    with tile.TileContext(nc) as tc:
        with (
            tc.tile_pool(name="const", bufs=1) as constp,
            tc.tile_pool(name="inp", bufs=1) as inp,
            tc.tile_pool(name="attn", bufs=8) as attnp,
            tc.tile_pool(name="outp", bufs=2) as outp,
            tc.tile_pool(name="rp", bufs=4) as rpp,
            tc.tile_pool(name="qk_ps", bufs=3, space="PSUM") as qkps,
            tc.tile_pool(name="pv_ps", bufs=2, space="PSUM") as pvps,
        ):
            # triangular causal mask: keep where qf - p >= 0 (one 128x128)
            cmask = constp.tile([128, KB], bf16)
            nc.gpsimd.memset(cmask[:], 1.0)
            nc.gpsimd.affine_select(
                out=cmask[:],
                in_=cmask[:],
                pattern=[[1, KB]],
                compare_op=mybir.AluOpType.is_ge,
                fill=0.0,
                base=0,
                channel_multiplier=-1,
            )

            qt = inp.tile([DK, S], bf16)
            kt = inp.tile([DK, NK, KB], bf16)
            vp = inp.tile([KB, NK, DK + 1], bf16)
            # Input DMAs are split across BOTH hardware DGE queues (scalar
            # + sync) and ordered to match processing order: the scalar DGE
            # carries just the first superblock's critical operands (96KB)
            # so the PE can start ~5us sooner; q streams superblock-by-
            # superblock in descending order on sync so each block lands
            # just before it is needed.  Output DMAs also ride sync and
            # overlap the compute instead of queueing behind one giant
            # q transfer.
            nc.scalar.dma_start(out=kt[:, 0:2], in_=kt_d[:, 0:2])
            nc.scalar.dma_start(out=qt[:, S - QB:S], in_=qt_d[:, S - QB:S])
            nc.sync.dma_start(out=vp[:, 0:4], in_=vp_d[:, 0:4])
            nc.sync.dma_start(out=kt[:, 2:8], in_=kt_d[:, 2:8])
            nc.sync.dma_start(out=vp[:, 4:16], in_=vp_d[:, 4:16])
            nc.sync.dma_start(out=kt[:, 8:32], in_=kt_d[:, 8:32])
            nc.sync.dma_start(out=vp[:, 16:32], in_=vp_d[:, 16:32])
            for blk in range(NQ - 2, -1, -1):
                nc.sync.dma_start(
                    out=qt[:, blk * QB:(blk + 1) * QB],
                    in_=qt_d[:, blk * QB:(blk + 1) * QB],
                )

            cost = {"s": 0.0, "v": 0.0}

            def emit_exp(dst, src, cols, forced=None):
                cs = cost["s"] + cols * RATE_S + OVH_S
                cv = cost["v"] + cols * RATE_V + OVH_V
                eng = forced or ("s" if cs < cv else "v")
                if eng == "s":
                    cost["s"] = cs
                    nc.scalar.activation(
                        out=dst, in_=src,
                        func=mybir.ActivationFunctionType.Exp,
                        scale=1.0 / DK,
                    )
                else:
                    cost["v"] = cost["v"] + cols * RATE_V + OVH_V
                    nc.vector._custom_dve(
                        exp_op, out=dst, in0=src,
                        s0=EXP_C0, s1=EXP_C1, imm2=EXP_C2,
                    )

            def emit_epilogue(I, pv):
                r4 = rpp.tile([128, 4], f32, tag="r")
                nc.vector.reciprocal(r4[:], pv[:, :, DK])
                cost["v"] += 280
                ot = outp.tile([128, 4, DK], f32, tag="ot")
                for t in range(4):
                    cs = cost["s"] + DK * RATE_S + 340
                    cv = cost["v"] + DK * RATE_V + 250
                    if cs < cv:
                        cost["s"] = cs
                        nc.scalar.mul(
                            ot[:, t, :], pv[:, t, 0:DK], r4[:, t:t + 1]
                        )
                    else:
                        cost["v"] = cv
                        nc.vector.tensor_scalar(
                            ot[:, t, :],
                            pv[:, t, 0:DK],
                            r4[:, t:t + 1],
                            None,
                            mybir.AluOpType.mult,
                        )
                nc.sync.dma_start(
                    out=out_d[I * QB:(I + 1) * QB].rearrange(
                        "(t p) d -> p t d", p=128
                    ),
                    in_=ot[:],
                )

            def emit_pv(I, j, at, pv, sb_left):
                # PSUM start_tensor_calc marks the whole 2KB bank as
                # pending-zero, so only the FIRST matmul into the bank may
                # carry start=True; the other q-blocks' first writes land on
                # still-pending bytes and become fresh writes automatically.
                for u in range(TR):
                    jj = j + u
                    qb0 = max(0, jj - 4 * I)
                    for qb in range(qb0, 4):
                        nc.tensor.matmul(
                            pv[:, qb, 0:DK + 1],
                            at[:, u, qb * KB:(qb + 1) * KB],
                            vp[:, jj, :],
                            start=(jj == 0 and qb == 0),
                            stop=(jj == 4 * I + qb),
                            skip_group_check=True,
                        )
                sb_left[I] -= 1
                if sb_left[I] == 0:
                    emit_epilogue(I, pv)

            pending = deque()
            sb_left = {}
            for I in reversed(range(NQ)):   # big superblocks first: deep
                # pipeline from the start, shallow ones drain at the end
                C = 4 * I + 4          # causal kv chunks for this superblock
                pv = pvps.tile([128, 4, 128], f32, tag="pv")
                sb_left[I] = C // 2
                for j in range(0, C, TR):
                    qk = qkps.tile([128, TR, QB], f32, tag="qk")
                    at = attnp.tile([128, TR, QB], bf16, tag="at")
                    # v0[u]: first valid q column for chunk j+u (cols below
                    # are fully causally masked and skipped everywhere)
                    v0 = [max(0, ((j + u) - 4 * I) * KB) for u in range(TR)]
                    for u in range(TR):
                        jj = j + u
                        nc.tensor.matmul(
                            qk[:, u, v0[u]:QB],
                            kt[:, jj, :],
                            qt[:, I * QB + v0[u]:(I + 1) * QB],
                            start=True, stop=True,
                        )
                    # exp: diagonal 128x128 triangles get the fused exp*mask
                    # DVE op; clean rectangles get plain exp, assigned to
                    # ScalarE or VectorE by the running-cost balancer. Clean
                    # pairs are split one chunk per engine so both engines
                    # work the SAME pair concurrently (halves the exp latency
                    # the PE pipeline has to hide).
                    if v0 == [0, 0]:
                        a = "s" if cost["s"] <= cost["v"] else "v"
                        b = "v" if a == "s" else "s"
                        emit_exp(at[:, 0, :], qk[:, 0, :], QB, forced=a)
                        emit_exp(at[:, 1, :], qk[:, 1, :], QB, forced=b)
                    else:
                        for u in range(TR):
                            jj = j + u
                            if jj >= 4 * I:
                                # plain exp over diag+tail in one slice; the
                                # idle GpSimd zeroes the masked triangle of
                                # the bf16 at tile afterwards.
                                emit_exp(
                                    at[:, u, v0[u]:QB],
                                    qk[:, u, v0[u]:QB],
                                    QB - v0[u],
                                )
                                nc.gpsimd.affine_select(
                                    out=at[:, u, v0[u]:v0[u] + KB],
                                    in_=at[:, u, v0[u]:v0[u] + KB],
                                    pattern=[[1, KB]],
                                    compare_op=mybir.AluOpType.is_ge,
                                    fill=0.0,
                                    base=0,
                                    channel_multiplier=-1,
                                )
                            else:
                                emit_exp(at[:, u, :], qk[:, u, :], QB)
                    pending.append((I, j, at, pv))
                    if len(pending) >= PIPE_DEPTH + PV_BURST:
                        for _ in range(PV_BURST):
                            emit_pv(*pending.popleft(), sb_left)
            while pending:
                emit_pv(*pending.popleft(), sb_left)

    nc.compile()
    return nc


def _get_nc():
    if "nc" not in _cache:
        _cache["nc"] = _build()
    return _cache["nc"]


def make_in_maps(q, k, v):
    bf = ml_dtypes.bfloat16
    q = np.asarray(q)
    k = np.asarray(k)
    v = np.asarray(v)
    in_maps = []
    for b in range(B):
        qtp = np.ascontiguousarray(q[b].T).astype(bf)         # [64, 4096]
        ktp = np.ascontiguousarray(
            k[b].T.reshape(DK, NK, KB)
        ).astype(bf)                                          # [64, 32, 128]
        vpk = np.empty((KB, NK, DK + 1), dtype=bf)
        vpk[:, :, 0:DK] = v[b].reshape(NK, KB, DK).transpose(1, 0, 2)
        vpk[:, :, DK] = 1.0
        in_maps.append({"qtp": qtp, "ktp": np.ascontiguousarray(ktp),
                        "vp": vpk})
    return in_maps


def kernel(q, k, v):
    from concourse.bass_utils import run_bass_kernel_spmd

    nc = _get_nc()
    in_maps = make_in_maps(q, k, v)
    res = run_bass_kernel_spmd(nc, in_maps, core_ids=list(range(B)))
    out = np.stack([np.asarray(res.results[i]["out"]) for i in range(B)], axis=0)
    return out.astype(np.float32)


# revision 21
# speedup vs baseline: 1.6108x; 1.6108x over previous
"""Causal attention head (B=8, S=4096, dk=64, scale=1/dk) on 8 TRN2 NeuronCores.

Data-parallel: batch b -> core b (no collectives). Per core, flash-style
causal attention with scores in [kv, q] orientation and output in [q, d]:
  - QK^T bf16, row-packed 2x via 64x128 PE tiling (chunk pairs at
    tile_position (0,0)/(64,0) execute CONCURRENTLY in separate PE row
    groups): q^T is duplicated into both partition halves, k^T chunks
    alternate halves. ~250ns per 2x512 score columns.
  - exp(x/64) on the [128, 2, 512] PSUM pair-tiles split between ScalarE
    (LUT exp, 1/dk scale folded in) and VectorE (custom single-pass DVE
    poly op), assigned by a running-cost balancer at measured rates.
    Diagonal 128x128 triangles use a fused exp*mask DVE op.
  - PV uses the exp'd scores (bf16 SBUF) as the STATIONARY operand and
    v (+ ones column for the denominator) as MOVING: out[q=128, 65] +=
    at[kv, qblk]^T @ vp[kv, 65]. This is weight-load bound (~55ns per
    128x128 score block) - same PE cost as the [d, q] orientation - but
    the output lands directly in [q, d]: no epilogue transposes and no
    PSUM->SBUF copy, saving ~7us of Vector/PE epilogue work.
  - PV emission is delayed PIPE_DEPTH chunk-pairs behind QK so the PE
    stays continuously busy (it must not stall or it drops out of the
    2.4GHz p-state); epilogue is reciprocal of the ones column + 4
    normalizes (scalar.mul/vector.tensor_scalar by running cost) + DMA.
All PSUM accumulation for a superblock lives in one 2KB bank; only the
first matmul into the bank carries start=True (start marks the whole
bank pending-zero, so interleaved per-qblock groups rely on pending
bytes making their first write a fresh write).
Superblocks are processed largest-first so the pipeline is deep from the
start and the shallow ones drain at the end.

Host-side shard packing per batch (layout only; all math is on-device):
  qtp [128, 4096] bf16 : q^T duplicated into both partition halves
  ktp [128, 16, 128] bf16 : k^T chunk 2m in partitions 0-63, 2m+1 in 64-127
  vp  [128, 32, 65] bf16 : v chunks (kv on partitions) + ones column
"""

import numpy as np
import ml_dtypes
from collections import deque

B, S, DK = 8, 4096, 64
QB = 512           # q superblock width (PSUM bank = 512 fp32)
KB = 128           # kv chunk (partition dim)
NK = S // KB       # 32 kv chunks
NQ = S // QB       # 8 q superblocks
TR = 2             # kv chunks per QK psum tile (2 PSUM banks, 3 bufs)
PIPE_DEPTH = 3     # chunk-pairs of PV delay behind QK on the PE stream
PV_BURST = 2       # pairs of PV drained back-to-back (fewer PE mode flips)

_cache = {}

# exp(x/64) ~= ((c0 + c1*x + c2*x^2)^2)^2)^2  (quadratic fit of exp(x/512)
# on |x|<=64, then 3 squarings). Max rel err ~7e-4 for |x|<=64.
EXP_C0, EXP_C1, EXP_C2 = 1.0, 0.001956942, 1.909212e-06

# greedy engine-balance cost model (ns): per-column rate + per-instr
# overhead, from measured hw traces (both engines run ~1x per column here)
RATE_S, OVH_S = 1.00, 180.0     # ScalarE activation
RATE_V, OVH_V = 1.15, 150.0     # VectorE custom DVE


def _pin_sha(op):
    import re

    for ver in ("v3",):
        try:
            op.compile(ver)
        except ValueError as e:
            m = re.search(r'uops_sha\["' + ver + r'"\]="([0-9a-f]+)"', str(e))
            if not m:
                raise
            op.uops_sha[ver] = m.group(1)
            op.compile(ver)


def _register_exp_ops():
    """Custom single-pass DVE ops:
    EXP_P8_ANT:  out = sq(sq(sq(c2*x^2 + c1*x + c0)))          ~ exp(x/64)
    EXP_P8M_ANT: out = sq(sq(sq(c2*x^2 + c1*x + c0))) * in1    fused mask
    """
    from concourse import dve_ops
    from concourse.dve_spec import Spec, Src0, Src1, C0, C1, C2, sq

    have = {o.name: o for o in dve_ops.OPS}
    if "EXP_P8_ANT" in have:
        return have["EXP_P8_ANT"], have["EXP_P8M_ANT"]
    poly = sq(sq(sq((Src0 * C2 + C1) * Src0 + C0)))

    def ref(in0, in1, s0, s1, imm2):
        return ((((in0 * imm2 + s1) * in0 + s0) ** 2) ** 2) ** 2

    spec_e = Spec(body=poly, reference=ref)
    spec_m = Spec(
        body=poly * Src1,
        reference=lambda in0, in1, s0, s1, imm2: ref(in0, in1, s0, s1, imm2)
        * in1,
    )
    ops = []
    for name, spec in (("EXP_P8_ANT", spec_e), ("EXP_P8M_ANT", spec_m)):
        op = dve_ops.DveOp(name, spec, subdim=False, uops_sha={})
        dve_ops.OPS.append(op)
        dve_ops.CUSTOM_DVE_SPECS[name] = spec
        dve_ops._SUB_OPCODE_FOR_NAME[name] = (
            max(dve_ops._SUB_OPCODE_FOR_NAME.values()) + 1
        )
        _pin_sha(op)
        ops.append(op)
    return ops[0], ops[1]


def _build():
    from concourse.bacc import Bacc
    from concourse import tile
    import concourse.mybir as mybir

    exp_op, expm_op = _register_exp_ops()

    f32 = mybir.dt.float32
    bf16 = mybir.dt.bfloat16

    nc = Bacc(None, target_bir_lowering=False)
    qt_d = nc.dram_tensor("qtp", [128, S], bf16, kind="ExternalInput")
    kt_d = nc.dram_tensor("ktp", [128, NK // 2, KB], bf16, kind="ExternalInput")
    vp_d = nc.dram_tensor("vp", [KB, NK, DK + 1], bf16, kind="ExternalInput")
    out_d = nc.dram_tensor("out", [S, DK], f32, kind="ExternalOutput")

    with tile.TileContext(nc) as tc:
        with (
            tc.tile_pool(name="const", bufs=1) as constp,
            tc.tile_pool(name="inp", bufs=1) as inp,
            tc.tile_pool(name="attn", bufs=8) as attnp,
            tc.tile_pool(name="outp", bufs=2) as outp,
            tc.tile_pool(name="rp", bufs=4) as rpp,
            tc.tile_pool(name="qk_ps", bufs=3, space="PSUM") as qkps,
            tc.tile_pool(name="pv_ps", bufs=2, space="PSUM") as pvps,
        ):
            # triangular causal mask: keep where qf - p >= 0 (one 128x128)
            cmask = constp.tile([128, KB], bf16)
            nc.gpsimd.memset(cmask[:], 1.0)
            nc.gpsimd.affine_select(
                out=cmask[:],
                in_=cmask[:],
                pattern=[[1, KB]],
                compare_op=mybir.AluOpType.is_ge,
                fill=0.0,
                base=0,
                channel_multiplier=-1,
            )

            qt = inp.tile([128, S], bf16)
            kt = inp.tile([128, NK // 2, KB], bf16)
            vp = inp.tile([KB, NK, DK + 1], bf16)
            # Input DMAs are split across BOTH hardware DGE queues (scalar
            # + sync) and ordered to match processing order: the scalar DGE
            # carries just the first superblock's critical operands (96KB)
            # so the PE can start ~5us sooner; q streams superblock-by-
            # superblock in descending order on sync so each block lands
            # just before it is needed.  Output DMAs also ride sync and
            # overlap the compute instead of queueing behind one giant
            # q transfer.
            nc.scalar.dma_start(out=kt[:, 0:1], in_=kt_d[:, 0:1])
            nc.scalar.dma_start(out=qt[:, S - QB:S], in_=qt_d[:, S - QB:S])
            nc.sync.dma_start(out=vp[:, 0:4], in_=vp_d[:, 0:4])
            nc.sync.dma_start(out=kt[:, 1:8], in_=kt_d[:, 1:8])
            nc.sync.dma_start(out=vp[:, 4:16], in_=vp_d[:, 4:16])
            nc.sync.dma_start(out=kt[:, 8:16], in_=kt_d[:, 8:16])
            nc.sync.dma_start(out=vp[:, 16:32], in_=vp_d[:, 16:32])
            for blk in range(NQ - 2, -1, -1):
                nc.sync.dma_start(
                    out=qt[:, blk * QB:(blk + 1) * QB],
                    in_=qt_d[:, blk * QB:(blk + 1) * QB],
                )

            cost = {"s": 0.0, "v": 0.0}

            def emit_exp(dst, src, cols, forced=None):
                cs = cost["s"] + cols * RATE_S + OVH_S
                cv = cost["v"] + cols * RATE_V + OVH_V
                eng = forced or ("s" if cs < cv else "v")
                if eng == "s":
                    cost["s"] = cs
                    nc.scalar.activation(
                        out=dst, in_=src,
                        func=mybir.ActivationFunctionType.Exp,
                        scale=1.0 / DK,
                    )
                else:
                    cost["v"] = cost["v"] + cols * RATE_V + OVH_V
                    nc.vector._custom_dve(
                        exp_op, out=dst, in0=src,
                        s0=EXP_C0, s1=EXP_C1, imm2=EXP_C2,
                    )

            def emit_epilogue(I, pv):
                r4 = rpp.tile([128, 4], f32, tag="r")
                nc.vector.reciprocal(r4[:], pv[:, :, DK])
                cost["v"] += 280
                ot = outp.tile([128, 4, DK], f32, tag="ot")
                for t in range(4):
                    cs = cost["s"] + DK * RATE_S + 340
                    cv = cost["v"] + DK * RATE_V + 250
                    if cs < cv:
                        cost["s"] = cs
                        nc.scalar.mul(
                            ot[:, t, :], pv[:, t, 0:DK], r4[:, t:t + 1]
                        )
                    else:
                        cost["v"] = cv
                        nc.vector.tensor_scalar(
                            ot[:, t, :],
                            pv[:, t, 0:DK],
                            r4[:, t:t + 1],
                            None,
                            mybir.AluOpType.mult,
                        )
                nc.sync.dma_start(
                    out=out_d[I * QB:(I + 1) * QB].rearrange(
                        "(t p) d -> p t d", p=128
                    ),
                    in_=ot[:],
                )

            def emit_pv(I, j, at, pv, sb_left):
                # PSUM start_tensor_calc marks the whole 2KB bank as
                # pending-zero, so only the FIRST matmul into the bank may
                # carry start=True; the other q-blocks' first writes land on
                # still-pending bytes and become fresh writes automatically.
                for u in range(TR):
                    jj = j + u
                    qb0 = max(0, jj - 4 * I)
                    for qb in range(qb0, 4):
                        nc.tensor.matmul(
                            pv[:, qb, 0:DK + 1],
                            at[:, u, qb * KB:(qb + 1) * KB],
                            vp[:, jj, :],
                            start=(jj == 0 and qb == 0),
                            stop=(jj == 4 * I + qb),
                            skip_group_check=True,
                        )
                sb_left[I] -= 1
                if sb_left[I] == 0:
                    emit_epilogue(I, pv)

            pending = deque()
            sb_left = {}
            for I in reversed(range(NQ)):   # big superblocks first: deep
                # pipeline from the start, shallow ones drain at the end
                C = 4 * I + 4          # causal kv chunks for this superblock
                pv = pvps.tile([128, 4, 128], f32, tag="pv")
                sb_left[I] = C // 2
                for j in range(0, C, TR):
                    qk = qkps.tile([128, TR, QB], f32, tag="qk")
                    at = attnp.tile([128, TR, QB], bf16, tag="at")
                    # v0[u]: first valid q column for chunk j+u (cols below
                    # are fully causally masked and skipped everywhere)
                    v0 = [max(0, ((j + u) - 4 * I) * KB) for u in range(TR)]
                    for u in range(TR):
                        jj = j + u
                        m, h = jj // 2, jj % 2
                        nc.tensor.matmul(
                            qk[:, u, v0[u]:QB],
                            kt[h * 64:(h + 1) * 64, m, :],
                            qt[h * 64:(h + 1) * 64,
                               I * QB + v0[u]:(I + 1) * QB],
                            start=True, stop=True,
                            tile_position=(64 * h, 0),
                        )
                    # exp: diagonal 128x128 triangles get the fused exp*mask
                    # DVE op; clean rectangles get plain exp, assigned to
                    # ScalarE or VectorE by the running-cost balancer. Clean
                    # pairs are split one chunk per engine so both engines
                    # work the SAME pair concurrently (halves the exp latency
                    # the PE pipeline has to hide).
                    if v0 == [0, 0]:
                        a = "s" if cost["s"] <= cost["v"] else "v"
                        b = "v" if a == "s" else "s"
                        emit_exp(at[:, 0, :], qk[:, 0, :], QB, forced=a)
                        emit_exp(at[:, 1, :], qk[:, 1, :], QB, forced=b)
                    else:
                        for u in range(TR):
                            jj = j + u
                            if jj >= 4 * I:
                                # plain exp over diag+tail in one slice; the
                                # idle GpSimd zeroes the masked triangle of
                                # the bf16 at tile afterwards.
                                emit_exp(
                                    at[:, u, v0[u]:QB],
                                    qk[:, u, v0[u]:QB],
                                    QB - v0[u],
                                )
                                nc.gpsimd.affine_select(
                                    out=at[:, u, v0[u]:v0[u] + KB],
                                    in_=at[:, u, v0[u]:v0[u] + KB],
                                    pattern=[[1, KB]],
                                    compare_op=mybir.AluOpType.is_ge,
                                    fill=0.0,
                                    base=0,
                                    channel_multiplier=-1,
                                )
                            else:
                                emit_exp(at[:, u, :], qk[:, u, :], QB)
                    pending.append((I, j, at, pv))
                    if len(pending) >= PIPE_DEPTH + PV_BURST:
                        for _ in range(PV_BURST):
                            emit_pv(*pending.popleft(), sb_left)
            while pending:
                emit_pv(*pending.popleft(), sb_left)

    nc.compile()
    return nc


def _get_nc():
    if "nc" not in _cache:
        _cache["nc"] = _build()
    return _cache["nc"]


def make_in_maps(q, k, v):
    bf = ml_dtypes.bfloat16
    q = np.asarray(q)
    k = np.asarray(k)
    v = np.asarray(v)
    in_maps = []
    for b in range(B):
        qt = np.ascontiguousarray(q[b].T).astype(bf)          # [64, 4096]
        qtp = np.concatenate([qt, qt], axis=0)                # [128, 4096]
        kt = np.ascontiguousarray(k[b].T).astype(bf)          # [64, 4096]
        ktc = kt.reshape(DK, NK, KB)                          # [64, 32, 128]
        ktp = np.empty((128, NK // 2, KB), dtype=bf)
        ktp[0:DK] = ktc[:, 0::2, :]
        ktp[DK:128] = ktc[:, 1::2, :]
        vpk = np.empty((KB, NK, DK + 1), dtype=bf)
        vpk[:, :, 0:DK] = v[b].reshape(NK, KB, DK).transpose(1, 0, 2)
        vpk[:, :, DK] = 1.0
        in_maps.append({"qtp": qtp, "ktp": np.ascontiguousarray(ktp),
                        "vp": vpk})
    return in_maps


def kernel(q, k, v):
    from concourse.bass_utils import run_bass_kernel_spmd

    nc = _get_nc()
    in_maps = make_in_maps(q, k, v)
    res = run_bass_kernel_spmd(nc, in_maps, core_ids=list(range(B)))
    out = np.stack([np.asarray(res.results[i]["out"]) for i in range(B)], axis=0)
    return out.astype(np.float32)
